# revision 51
# baseline (speedup 1.0000x reference)
"""Trainium2 Bass kernel for Conv2D (1x1) multi-head attention block.

Reference computation (per batch image of [64, 64, 512] = [N=4096, C=512]):
    x  = GroupNorm(inputs, G=32, eps=1e-6) * gamma + beta
    q, k, v = x @ wq + bq, x @ wk + bk, x @ wv + bv      (1x1 convs)
    scores  = (q / sqrt(C)) @ k^T                         [N, N]
    out     = softmax(scores) @ v @ wo + bo + inputs

Sharding: 8 cores = 2 batches x 4 query-quarters.  Each core holds the full
image of its batch (full-attention K/V) and produces the output rows of its
query quarter.  No collectives.

Division of labor: the host does all input-independent weight algebra plus
the GroupNorm statistics (a 2x32-number reduction) and precision/layout
prep; the device runs every activation GEMM: the query projection
u = W2^T (a.x_q) + c2 (W2 = Wq Wk^T), the full N x N attention
(scores, softmax, attn.V), and the output projection via W3 = Wv Wo.

  - GroupNorm folds: a = gamma*rstd, b = beta - mean*a.  The host ships
    xa = a.x pre-scaled and cast to fp8_e4m3 in BOTH layouts the PE needs:
    channel-pair tiles xat8 [128, 2, N] (scores lhsT / stats-free) and
    pixel-pair tiles xnat8 (attn.V lhsT).  All additive GN/bias terms either
    cancel in softmax (key-side constants), fold into c2 (query side,
    c2 = Wk (Wq^T b + bq)), or ride through attention as constants into the
    residual (V side: res16 = fp16(x + (b Wv + bv) Wo + bo)).
  - Every matmul runs in fp8 DoubleRow mode (256-deep contraction per
    instruction).  Weights ship as fp8 scaled by S (=16); the 1/S is
    recovered in PSUM->SBUF epilogues.
  - Scores are computed per 128-key tile as scores^T[k, q]; exp runs on the
    scalar engine with scale 1/sqrt(C) and bias -2 (softmax shift
    invariance; keeps exp outputs inside fp8's +-240 range) writing fp8
    probs pairs.  attn^T accumulates over key-pair tiles in PSUM.  The
    kernel is software-pipelined: attnV of pair g-1 issues between the
    scores and exps of pair g.  The steady-state group is exactly 8
    back-to-back PE matmuls (216 ns each: 4 scores + 4 attnV) with the two
    exps hidden under them, so softmax denominators go to the otherwise
    idle DVE: probs accumulate into an f16 tile, reduced at chunk end by
    N=1 matmuls straight into per-query-partition layout (no transposes).
  - V is never materialized and neither is attn: y = xa^T probs^T
    accumulates in PSUM, then out rows = (ATS.y)^T (S.W3) with
    W3 = Wv Wo host-folded to fp8 -- one GEMM instead of two, and one
    fewer fp8 requantization.  fin = ops * recq + res is a single fused
    scalar_tensor_tensor per query tile (the scalar engine does only exps
    plus the final chunk's z8 scaling, so its Exp table is never evicted).
  - DMA: the ~16 hardware queues stripe every transfer, so completion
    order is issue order and per-row burst size is what matters; all
    layouts are chunk-major (1-2 KB contiguous rows) and the query
    projection's dependencies are issued first.  Epilogue work of chunk
    k is carried as thunks injected one-per-group into chunk k+1's score
    stream (chunk 1's query projection rides chunk 0 the same way), so
    the in-order PE queue never stalls on cross-engine round trips; each
    out-projection gets its own PSUM bank in the drain, and the fin pool
    is deep enough that output DMAs overlap.  f32r warm matmuls bridge
    the launch window and the drain so the HAM clock stays up.
"""

import sys

sys.path.insert(0, "/opt/trn_rl_repo")

from contextlib import ExitStack

import numpy as np

import concourse.bacc as bacc
import concourse.tile as tile
from concourse import mybir
from concourse.bass_utils import run_bass_kernel_spmd

# Problem shape (hardcoded; kernel.py must be self-contained).
B, HH, WW, C = 2, 64, 64, 512
N = HH * WW          # 4096 pixels per image
G = 32               # groupnorm groups
GS = C // G          # 16 channels per group
EPS = 1e-6
P = 128              # partitions
CT = C // P          # 4 channel tiles
CP = CT // 2         # 2 channel-pair tiles
NT = N // P          # 32 pixel tiles per image
NP2 = NT // 2        # 16 pixel-pair tiles
NCORES = 8
QS = N // 4          # 1024 query rows per core
QTILES = QS // P     # 8 query tiles per core
QCH = QS // 512      # 2 query chunks per core

S2 = 16.0            # fp8 scale for W2 = Wq @ Wk^T (host-precomputed)
S3 = 16.0            # fp8 scale for W3 = Wv @ Wo (host-precomputed)
ATS = 0.125          # unnormalized-attn fp8 scale (|attn_u| < ~800 -> <100)
ISQ = 1.0 / float(np.sqrt(float(C)))
SHIFT = -2.0         # exp(s*ISQ + SHIFT): keeps probs < 240 (fp8e4 max)

F32 = mybir.dt.float32
F16 = mybir.dt.float16
BF16 = mybir.dt.bfloat16
FP8 = mybir.dt.float8e4
AF = mybir.ActivationFunctionType
ALU = mybir.AluOpType
DR = mybir.MatmulPerfMode.DoubleRow

_NC_CACHE = None


def _build():
    nc = bacc.Bacc(None, target_bir_lowering=False, debug=False)

    xat8_d = [nc.dram_tensor(f"xat8p{g}", [8, P, 2, 512], FP8,
                              kind="ExternalInput") for g in range(CP)]
    xnat_d = nc.dram_tensor("xnat8", [NP2, P, 2, C], FP8, kind="ExternalInput")
    # query-quarter columns of xat, chunk-major so each DMA slice is
    # row-contiguous; pairs stacked along dim 2 as (g, i) -> 2g+i
    xaq8_d = nc.dram_tensor("xaq8", [P, QCH, 2 * CP, 512], FP8,
                            kind="ExternalInput")
    w2T8_d = nc.dram_tensor("w2T8c", [P, 2, CP * C], FP8,
                            kind="ExternalInput")
    w38_d = nc.dram_tensor("w38c", [P, 2, CP * C], FP8,
                           kind="ExternalInput")
    c2_d = nc.dram_tensor("c2", [1, C], F32, kind="ExternalInput")
    res_d = nc.dram_tensor("res16", [QS, C], F16, kind="ExternalInput")
    one_d = nc.dram_tensor("one11", [1, 1], F32, kind="ExternalInput")
    out_d = nc.dram_tensor("out", [QS, C], BF16, kind="ExternalOutput")

    with tile.TileContext(nc) as tc, ExitStack() as top:
        consts = top.enter_context(tc.tile_pool(name="consts", bufs=1))
        pxt = top.enter_context(tc.tile_pool(name="pxt", bufs=1))
        pv = top.enter_context(tc.tile_pool(name="pv", bufs=1))
        pq = top.enter_context(tc.tile_pool(name="pq", bufs=1))
        pres = top.enter_context(tc.tile_pool(name="pres", bufs=1))
        pmisc = top.enter_context(tc.tile_pool(name="pmisc", bufs=1))
        pe = top.enter_context(tc.tile_pool(name="pe", bufs=5))
        pef = top.enter_context(tc.tile_pool(name="pef", bufs=4))
        # PSUM: sc 2 + at 4 + rows 1 + ops 1 = 8 banks
        pss = top.enter_context(tc.tile_pool(name="pss", bufs=2, space="PSUM"))
        psat = top.enter_context(tc.tile_pool(name="psat", bufs=1, space="PSUM"))
        psr = top.enter_context(tc.tile_pool(name="psr", bufs=1, space="PSUM"))
        pso = top.enter_context(tc.tile_pool(name="pso", bufs=1, space="PSUM"))

        # ---------- consts (no DMA dependencies) ----------
        one11 = consts.tile([1, 1], F32, name="one11")
        nc.sync.dma_start(out=one11, in_=one_d[:])
        # rowsum weight: folds the ATS*S3 denominator scale into the
        # partition reduction
        sumw = consts.tile([P, 1], F16, name="sumw")
        nc.vector.memset(sumw, ATS * S3)
        sumw8 = consts.tile([P, 1], FP8, name="sumw8")
        nc.vector.memset(sumw8, ATS * S3)
        ebias = consts.tile([P, 1], F32, name="ebias")
        nc.vector.memset(ebias, SHIFT)
        warm32 = pmisc.tile([P, 512], F32, name="warm32")
        nc.vector.memset(warm32, 1.0)
        warmr = pmisc.tile([P, 512], mybir.dt.float32r, name="warmr")
        nc.vector.tensor_copy(warmr, warm32)

        # ---------- resident tensors ----------
        xat8 = [pxt.tile([P, 8, 2, 512], FP8, name=f"xat8_{g}",
                         tag=f"xat8_{g}") for g in range(CP)]
        xnat = [pv.tile([P, 2, C], FP8, name=f"xnat_{g}", tag=f"xnat_{g}")
                for g in range(NP2)]
        xaq8 = pq.tile([P, QCH, 2 * CP, 512], FP8, name="xaq8", tag="xaq8")
        u8 = [pq.tile([P, 2, QS], FP8, name=f"u8_{g}", tag=f"u8_{g}")
              for g in range(CP)]
        w2T8 = pq.tile([P, 2, CP * C], FP8, name="w2T8", tag="w2T8")
        w38 = pq.tile([P, 2, CP * C], FP8, name="w38", tag="w38")
        res16 = [pres.tile([P, C], F16, name=f"res16_{i}", tag=f"res_{i}")
                 for i in range(QTILES)]
        # c2 ships as a single row (one DMA packet; a [P,1]-shaped load is
        # 128 four-byte packets that clog the startup DMA window) and is
        # transposed on-device via tiny one11 matmuls.
        c2row = consts.tile([1, C], F32, name="c2row")
        nc.sync.dma_start(out=c2row, in_=c2_d[:])

        # ---------- DMA issue order: the ~16 hardware queues are assigned
        # round-robin in call order and run CONCURRENTLY, so a transfer's
        # priority is its share of queues, not its position.  Slice the
        # u8-projection dependencies (w2T8 + xaq8) along the PARTITION dim
        # (keeps per-row bursts contiguous; free-dim slicing fragments rows
        # into tiny bursts) so they own most of the queues and land first;
        # everything else follows in consumption order, residuals last. ----
        nc.sync.dma_start(out=w2T8, in_=w2T8_d[:])
        for ch2 in range(QCH):
            nc.sync.dma_start(out=xaq8[:, ch2], in_=xaq8_d[:, ch2])
        nc.sync.dma_start(out=w38, in_=w38_d[:])
        for ch in range(8):          # 512-pixel column chunks, kt-major
            for g in range(CP):
                nc.sync.dma_start(out=xat8[g][:, ch], in_=xat8_d[g][ch])
            nc.sync.dma_start(out=xnat[2 * ch], in_=xnat_d[2 * ch])
            nc.sync.dma_start(out=xnat[2 * ch + 1], in_=xnat_d[2 * ch + 1])
        for i in range(QTILES):
            nc.sync.dma_start(out=res16[i], in_=res_d[i * P:(i + 1) * P, :])

        def keep_warm(n):
            # Full-width f32r matmuls keep the HAM clock at full rate while
            # the PE would otherwise idle (low-toggle fp8 matmuls don't
            # register enough activity and the whole core drops to half
            # clock, with ~10us of hysteresis).
            for _ in range(n):
                wps = pss.tile([P, 512], F32, name="wps", tag="sc")
                nc.tensor.matmul(wps, lhsT=warmr[:, 0:P], rhs=warmr,
                                 start=True, stop=True)

        keep_warm(2)

        # Preload the scalar engine's Exp activation table (costs 1.3us;
        # otherwise it lands at the first real exp, right at attention
        # start).  The scalar engine runs nothing but Exp, so the table is
        # never evicted.
        expwarm = pmisc.tile([P, 1], F32, name="expwarm")
        nc.scalar.activation(expwarm, ebias, AF.Exp, bias=ebias, scale=1.0)

        # c2 row -> per-partition [P, 4] via tiny transpose matmuls
        c2_ps = pso.tile([P, 8], F32, name="c2_ps", tag="ops")
        for ct in range(CT):
            nc.tensor.matmul(c2_ps[:, ct:ct + 1],
                             lhsT=c2row[0:1, ct * P:(ct + 1) * P],
                             rhs=one11, start=True, stop=True)
        c24 = pmisc.tile([P, 4], F32, name="c24")
        nc.vector.tensor_copy(c24, c2_ps[:, 0:4])
        keep_warm(1)

        # ---- query projection: u = W2^T xa_q / S2 + c2, fp8 ----
        # ps lives in the pso bank: chunk 1's projections are injected into
        # chunk 0's score stream, and borrowing the sc rotation there would
        # stall the scores on the exp reads.
        def u8_proj(ch2, ci_t):
            if ci_t % 2 == 0:
                ps = pso.tile([P, 512], F32, name="ups", tag="ops")
            else:
                ps = psr.tile([P, 512], F32, name="ups2", tag="rows")
            for gq in range(CP):
                nc.tensor.matmul(
                    ps,
                    lhsT=w2T8[:, :, gq * C + ci_t * P:
                              gq * C + (ci_t + 1) * P],
                    rhs=xaq8[:, ch2, 2 * gq:2 * gq + 2, :],
                    start=(gq == 0), stop=(gq == CP - 1),
                    perf_mode=DR)
            og, oi = divmod(ci_t, 2)
            nc.vector.tensor_scalar(
                u8[og][:, oi, ch2 * 512:(ch2 + 1) * 512],
                ps, 1.0 / S2, c24[:, ci_t:ci_t + 1], ALU.mult, ALU.add)

        for ci_t in range(CT):
            u8_proj(0, ci_t)

        # ---- attention + output projection ----
        # ep_carry: thunks of deferred PE/DVE work (the previous chunk's
        # epilogue, or the second chunk's query projection) injected one
        # per score group so the in-order PE queue never stalls on the
        # DVE-paced epilogue.  The chunk's first group has no attn_v, so
        # multi-matmul thunks land there for free.
        ep_carry = [(lambda ci_t=ci_t: u8_proj(1, ci_t)) for ci_t in range(CT)]

        def attn_v(g, probs, at_ps):
            for co in range(CT):
                nc.tensor.matmul(
                    at_ps[co],
                    lhsT=xnat[g][:, :, co * P:(co + 1) * P],
                    rhs=probs,
                    start=(g == 0), stop=(g == NP2 - 1),
                    perf_mode=DR)

        for qc in range(QCH):
            at_ps = [psat.tile([P, 512], F32, name=f"at{co}",
                               tag=f"at{co}") for co in range(CT)]
            # Softmax denominators, split: even groups' probs accumulate on
            # the DVE into acc (f16), odd groups' via a ones-matmul on the
            # PE into rows_ps.  All-DVE backs up the DVE queue and stalls
            # the probs rotation; all-PE costs a 9th matmul every group.
            acc = pe.tile([P, 2, 512], F16, name="acc", tag="acc")

            prev = None
            for g in range(NP2):
                scs = []
                for j in range(2):
                    kt_i = 2 * g + j
                    sc = pss.tile([P, 512], F32, name="sc", tag="sc")
                    kch, kof = divmod(kt_i, 4)
                    for c in range(CP):
                        nc.tensor.matmul(
                            sc,
                            lhsT=xat8[c][:, kch, :,
                                         kof * P:(kof + 1) * P],
                            rhs=u8[c][:, :, qc * 512:(qc + 1) * 512],
                            start=(c == 0), stop=(c == CP - 1),
                            perf_mode=DR)
                    scs.append(sc)
                if prev is not None:
                    attn_v(g - 1, prev, at_ps)
                if ep_carry:
                    ep_carry.pop(0)()
                probs = pe.tile([P, 2, 512], FP8, name="probs", tag="probs")
                for j in range(2):
                    nc.scalar.activation(probs[:, j, :], scs[j], AF.Exp,
                                         bias=ebias, scale=ISQ)
                if g == 0:
                    nc.vector.tensor_copy(acc, probs)
                elif g < NP2 - 1:
                    # group 15 is folded into do_rows directly from its
                    # probs tile, so the denominators never wait on the
                    # final DVE accumulate
                    nc.vector.tensor_add(acc, acc, probs)
                prev = probs
            attn_v(NP2 - 1, prev, at_ps)

            # ---- chunk epilogue ----
            last = qc == QCH - 1
            z8 = [pe.tile([P, 2, 512], FP8, name=f"z8_{zg}", tag=f"z8_{zg}")
                  for zg in range(CP)]
            recq4 = pe.tile([P, 4], F32, name="recq4", tag="recq4")

            def do_rows(acc=acc, recq4=recq4, p15=prev):
                # denominators directly in partition layout: one N=1 matmul
                # per (tile, pair) over the f16 accumulator (groups 0-14)
                # plus the last group's fp8 probs -- no [32, 512] reduction,
                # no transposes, one reciprocal
                rq_ps = pso.tile([P, 8], F32, name="rq_ps", tag="ops")
                for qt in range(4):
                    for i in range(2):
                        nc.tensor.matmul(
                            rq_ps[:, qt:qt + 1],
                            lhsT=acc[:, i, qt * P:(qt + 1) * P],
                            rhs=sumw, start=(i == 0), stop=False)
                    for i in range(2):
                        nc.tensor.matmul(
                            rq_ps[:, qt:qt + 1],
                            lhsT=p15[:, i, qt * P:(qt + 1) * P],
                            rhs=sumw8, start=False, stop=(i == 1))
                nc.vector.reciprocal(recq4, rq_ps[:, 0:4])

            def mk_oproj(qt, qc=qc, z8=z8, recq4=recq4, opool=None,
                         fin_eng=None):
                def thunk():
                    if opool is None:
                        ops = pso.tile([P, C], F32, name="ops", tag="ops")
                    else:
                        ops = opool[0].tile([P, C], F32, name="ops",
                                            tag=opool[1])
                    for zg in range(CP):
                        nc.tensor.matmul(
                            ops, lhsT=z8[zg][:, :, qt * P:(qt + 1) * P],
                            rhs=w38[:, :, zg * C:(zg + 1) * C],
                            start=(zg == 0),
                            stop=(zg == CP - 1), perf_mode=DR)
                    fin2 = pef.tile([P, C], BF16, name="fin2", tag="fin2")
                    (fin_eng or nc.vector).scalar_tensor_tensor(
                        fin2, ops, recq4[:, qt:qt + 1], res16[qc * 4 + qt],
                        ALU.mult, ALU.add)
                    r0 = (qc * 4 + qt) * P
                    nc.sync.dma_start(out=out_d[r0:r0 + P, :], in_=fin2)
                return thunk

            if qc < QCH - 1:
                for ci_t in range(CT):
                    og, oi = divmod(ci_t, 2)
                    nc.vector.tensor_scalar_mul(z8[og][:, oi, :],
                                                at_ps[ci_t], ATS)
                ep_carry = [do_rows, mk_oproj(0, opool=(psr, "rows"))] + \
                    [mk_oproj(qt) for qt in range(1, 4)]
            else:
                # Last chunk: no next score stream to hide behind.  Spread
                # the out-projections over the now-free sc/at banks so they
                # run back-to-back, with warm matmuls keeping the clock up
                # through the fin/DMA drain.  z8 rides the freed scalar
                # engine; its first op takes a zero bias derived from recq4
                # -- a fake dependency that makes the denominator matmuls
                # transitively critical, so the dependency-order scheduler
                # runs them before the out-projection chain instead of
                # after it (which stalls the fins on the reciprocal).
                do_rows()
                sats = pe.tile([P, 1], F32, name="sats", tag="sats")
                nc.vector.tensor_scalar(sats, recq4[:, 0:1], 0.0, ATS,
                                        ALU.mult, ALU.add)
                # z8 split scalar/DVE: with the final probs accumulate
                # gone (group 15 rides do_rows), the DVE is idle here, so
                # the two chains of two finish in half the time
                for ci_t in range(CT):
                    og, oi = divmod(ci_t, 2)
                    if ci_t == 0:
                        nc.scalar.activation(z8[og][:, oi, :], at_ps[ci_t],
                                             AF.Copy, bias=0.0, scale=sats)
                    elif ci_t == 1:
                        nc.scalar.mul(z8[og][:, oi, :], at_ps[ci_t], ATS)
                    else:
                        nc.vector.tensor_scalar_mul(z8[og][:, oi, :],
                                                    at_ps[ci_t], ATS)
                keep_warm(2)
                mk_oproj(0, opool=(psr, "rows"))()
                mk_oproj(1, opool=(pss, "sc"))()
                keep_warm(2)
                mk_oproj(2, opool=(pss, "sc"))()
                mk_oproj(3, opool=(psat, "at0"))()
                keep_warm(2)

    nc.compile()
    return nc


def _make_in_maps(inputs):
    import ml_dtypes
    FP8NP = ml_dtypes.float8_e4m3
    x = np.ascontiguousarray(np.asarray(inputs["inputs"], dtype=np.float32))
    xf = x.reshape(B, N, C)
    gamma = np.asarray(inputs["gn_gamma"], np.float32)
    beta = np.asarray(inputs["gn_beta"], np.float32)
    wq = np.asarray(inputs["wq"], np.float32)
    wk = np.asarray(inputs["wk"], np.float32)
    wv = np.asarray(inputs["wv"], np.float32)
    wo = np.asarray(inputs["wo"], np.float32)
    bq = np.asarray(inputs["bq"], np.float32)
    bv = np.asarray(inputs["bv"], np.float32)
    bo = np.asarray(inputs["bo"], np.float32)

    shared = {"one11": np.ones((1, 1), np.float32)}
    w2T = (wq @ wk.T) * S2
    w2T8p = w2T.astype(FP8NP).reshape(CP, 2, P, C).transpose(2, 1, 0, 3)
    shared["w2T8c"] = np.ascontiguousarray(
        w2T8p.reshape(P, 2, CP * C))
    w3 = (wv @ wo) * S3
    w38p = w3.astype(FP8NP).reshape(CP, 2, P, C).transpose(2, 1, 0, 3)
    shared["w38c"] = np.ascontiguousarray(w38p.reshape(P, 2, CP * C))

    # Per-batch GroupNorm folds.
    per_b = []
    for b in range(B):
        xg = xf[b].reshape(N, G, GS)
        mean = xg.mean(axis=(0, 2))
        var = xg.var(axis=(0, 2))
        a = (gamma.reshape(G, GS) / np.sqrt(var[:, None] + EPS)).reshape(C)
        bvec = beta - np.repeat(mean, GS) * a
        xa = xf[b] * a                               # [N, C]
        xa8 = xa.astype(FP8NP)
        # channel-pair tiles: xat8p[g][p, i, n] = xa^T[g*256 + i*128 + p, n]
        xaT = np.ascontiguousarray(xa8.T)            # [C, N] fp8
        xat_pairs = [np.ascontiguousarray(
            xaT.reshape(CP, 2, P, N)[g].transpose(1, 0, 2))
            for g in range(CP)]                          # [P, 2, N]
        # chunk-major DMA view: [8, P, 2, 512]
        xat_cm = [np.ascontiguousarray(
            t.reshape(P, 2, 8, 512).transpose(2, 0, 1, 3))
            for t in xat_pairs]
        # pixel-pair tiles: xnat8[gk][p, ik, c]
        xnat = np.ascontiguousarray(
            xa8.reshape(NP2, 2, P, C).transpose(0, 2, 1, 3))
        c2v = wk @ (bvec @ wq + bq)                  # [C]
        bo_eff = (bvec @ wv + bv) @ wo + bo          # [C]
        per_b.append((xat_pairs, xat_cm, xnat,
                      np.ascontiguousarray(c2v.astype(np.float32)
                                           .reshape(1, C)), bo_eff))

    in_maps = []
    for core in range(NCORES):
        b, qq = divmod(core, 4)
        xat_pairs, xat_cm, xnat, c2v, bo_eff = per_b[b]
        m = dict(shared)
        for g in range(CP):
            m[f"xat8p{g}"] = xat_cm[g]
        m["xnat8"] = xnat
        m["c2"] = c2v
        # [P, 2*CP, QS] -> chunk-major [P, QCH, 2*CP, 512]
        xq = np.concatenate(
            [xat_pairs[g][:, :, qq * QS:(qq + 1) * QS] for g in range(CP)],
            axis=1)
        m["xaq8"] = np.ascontiguousarray(
            xq.reshape(P, 2 * CP, QCH, 512).transpose(0, 2, 1, 3))
        m["res16"] = np.ascontiguousarray(
            (xf[b, qq * QS:(qq + 1) * QS, :] + bo_eff).astype(np.float16))
        in_maps.append(m)
    return in_maps


def _assemble(results):
    out = np.empty((B, N, C), dtype=np.float32)
    for core in range(NCORES):
        b, qq = divmod(core, 4)
        out[b, qq * QS:(qq + 1) * QS, :] = results[core]["out"]
    return out.reshape(B, HH, WW, C)


def kernel(**inputs):
    global _NC_CACHE
    if _NC_CACHE is None:
        _NC_CACHE = _build()
    in_maps = _make_in_maps(inputs)
    res = run_bass_kernel_spmd(_NC_CACHE, in_maps, list(range(NCORES)))
    return _assemble(res.results)


def _install_ntff_shim():
    """The agent image's antenv lacks axon_hooks; provide it so
    run_bass_kernel_spmd(trace=True) can NTFF-profile through axon."""
    import types
    import antenv
    if "antenv.axon_hooks" in sys.modules:
        return
    mod = types.ModuleType("antenv.axon_hooks")
    mod._hook = None

    def set_axon_ntff_profile_hook(h):
        mod._hook = h

    def get_axon_ntff_profile_hook():
        return mod._hook

    mod.set_axon_ntff_profile_hook = set_axon_ntff_profile_hook
    mod.get_axon_ntff_profile_hook = get_axon_ntff_profile_hook
    sys.modules["antenv.axon_hooks"] = mod
    antenv.axon_hooks = mod
    sys.path.insert(0, "/root/.axon_site")
    from trn_agent_boot.trn_boot import _ntff_profile_via_ctypes
    hook = _ntff_profile_via_ctypes("/opt/axon/libaxon_pjrt.so")
    set_axon_ntff_profile_hook(hook)


def run_traced(inputs, trace_kwargs=None):
    """Traced run for profiling: returns (BassKernelResults, tmpdir)."""
    global _NC_CACHE
    if _NC_CACHE is None:
        _NC_CACHE = _build()
    import tempfile
    _install_ntff_shim()
    in_maps = _make_in_maps(inputs)
    tmpdir = tempfile.mkdtemp(prefix="trace_")
    res = run_bass_kernel_spmd(_NC_CACHE, in_maps, list(range(NCORES)),
                               trace=True, tmpdir=tmpdir,
                               trace_kwargs=trace_kwargs or {})
    return res, tmpdir


# revision 52
# speedup vs baseline: 1.0416x; 1.0416x over previous
"""Trainium2 Bass kernel for Conv2D (1x1) multi-head attention block.

Reference computation (per batch image of [64, 64, 512] = [N=4096, C=512]):
    x  = GroupNorm(inputs, G=32, eps=1e-6) * gamma + beta
    q, k, v = x @ wq + bq, x @ wk + bk, x @ wv + bv      (1x1 convs)
    scores  = (q / sqrt(C)) @ k^T                         [N, N]
    out     = softmax(scores) @ v @ wo + bo + inputs

Sharding: 8 cores = 2 batches x 4 query-quarters.  Each core holds the full
image of its batch (full-attention K/V) and produces the output rows of its
query quarter.  No collectives.

Division of labor: the host does all input-independent weight algebra plus
the GroupNorm statistics (a 2x32-number reduction) and precision/layout
prep; the device runs every activation GEMM: the query projection
u = W2^T (a.x_q) + c2 (W2 = Wq Wk^T), the full N x N attention
(scores, softmax, attn.V), and the output projection via W3 = Wv Wo.

  - GroupNorm folds: a = gamma*rstd, b = beta - mean*a.  The host ships
    xa = a.x pre-scaled and cast to fp8_e4m3 in BOTH layouts the PE needs:
    channel-pair tiles xat8 [128, 2, N] (scores lhsT / stats-free) and
    pixel-pair tiles xnat8 (attn.V lhsT).  All additive GN/bias terms either
    cancel in softmax (key-side constants), fold into c2 (query side,
    c2 = Wk (Wq^T b + bq)), or ride through attention as constants into the
    residual (V side: res16 = fp16(x + (b Wv + bv) Wo + bo)).
  - Every matmul runs in fp8 DoubleRow mode (256-deep contraction per
    instruction).  Weights ship as fp8 scaled by S (=16); the 1/S is
    recovered in PSUM->SBUF epilogues.
  - Scores are computed per 128-key tile as scores^T[k, q]; exp runs on the
    scalar engine with scale 1/sqrt(C) and bias -2 (softmax shift
    invariance; keeps exp outputs inside fp8's +-240 range) writing fp8
    probs pairs.  attn^T accumulates over key-pair tiles in PSUM.  The
    kernel is software-pipelined: attnV of pair g-1 issues between the
    scores and exps of pair g.  The steady-state group is exactly 8
    back-to-back PE matmuls (216 ns each: 4 scores + 4 attnV) with the two
    exps hidden under them, so softmax denominators go to the otherwise
    idle DVE: probs accumulate into an f16 tile, reduced at chunk end by
    N=1 matmuls straight into per-query-partition layout (no transposes).
  - V is never materialized and neither is attn: y = xa^T probs^T
    accumulates in PSUM, then out rows = (ATS.y)^T (S.W3) with
    W3 = Wv Wo host-folded to fp8 -- one GEMM instead of two, and one
    fewer fp8 requantization.  fin = ops * recq + res is a single fused
    scalar_tensor_tensor per query tile (the scalar engine does only exps
    plus the final chunk's z8 scaling, so its Exp table is never evicted).
  - DMA: the ~16 hardware queues stripe every transfer, so completion
    order is issue order and per-row burst size is what matters; all
    layouts are chunk-major (1-2 KB contiguous rows) and the query
    projection's dependencies are issued first.  Epilogue work of chunk
    k is carried as thunks injected one-per-group into chunk k+1's score
    stream (chunk 1's query projection rides chunk 0 the same way), so
    the in-order PE queue never stalls on cross-engine round trips; each
    out-projection gets its own PSUM bank in the drain, and the fin pool
    is deep enough that output DMAs overlap.  f32r warm matmuls bridge
    the launch window and the drain so the HAM clock stays up.
"""

import sys

sys.path.insert(0, "/opt/trn_rl_repo")

from contextlib import ExitStack

import numpy as np

import concourse.bacc as bacc
import concourse.tile as tile
from concourse import mybir
from concourse.bass_utils import run_bass_kernel_spmd

# Problem shape (hardcoded; kernel.py must be self-contained).
B, HH, WW, C = 2, 64, 64, 512
N = HH * WW          # 4096 pixels per image
G = 32               # groupnorm groups
GS = C // G          # 16 channels per group
EPS = 1e-6
P = 128              # partitions
CT = C // P          # 4 channel tiles
CP = CT // 2         # 2 channel-pair tiles
NT = N // P          # 32 pixel tiles per image
NP2 = NT // 2        # 16 pixel-pair tiles
NCORES = 8
QS = N // 4          # 1024 query rows per core
QTILES = QS // P     # 8 query tiles per core
QCH = QS // 512      # 2 query chunks per core

S2 = 16.0            # fp8 scale for W2 = Wq @ Wk^T (host-precomputed)
S3 = 16.0            # fp8 scale for W3 = Wv @ Wo (host-precomputed)
ATS = 0.125          # unnormalized-attn fp8 scale (|attn_u| < ~800 -> <100)
ISQ = 1.0 / float(np.sqrt(float(C)))
SHIFT = -2.0         # exp(s*ISQ + SHIFT): keeps probs < 240 (fp8e4 max)

F32 = mybir.dt.float32
F16 = mybir.dt.float16
BF16 = mybir.dt.bfloat16
FP8 = mybir.dt.float8e4
AF = mybir.ActivationFunctionType
ALU = mybir.AluOpType
DR = mybir.MatmulPerfMode.DoubleRow

_NC_CACHE = None


def _build():
    nc = bacc.Bacc(None, target_bir_lowering=False, debug=False)

    xat8_d = [nc.dram_tensor(f"xat8p{g}", [8, P, 2, 512], FP8,
                              kind="ExternalInput") for g in range(CP)]
    xnat_d = nc.dram_tensor("xnat8", [NP2, P, 2, C], FP8, kind="ExternalInput")
    # query-quarter columns of xat, chunk-major so each DMA slice is
    # row-contiguous; pairs stacked along dim 2 as (g, i) -> 2g+i
    xaq8_d = nc.dram_tensor("xaq8", [P, QCH, 2 * CP, 512], FP8,
                            kind="ExternalInput")
    w2T8_d = nc.dram_tensor("w2T8c", [P, 2, CP * C], FP8,
                            kind="ExternalInput")
    w38_d = nc.dram_tensor("w38c", [P, 2, CP * C], FP8,
                           kind="ExternalInput")
    c2_d = nc.dram_tensor("c2", [1, C], F32, kind="ExternalInput")
    res_d = nc.dram_tensor("res16", [QS, C], F16, kind="ExternalInput")
    one_d = nc.dram_tensor("one11", [1, 1], F32, kind="ExternalInput")
    out_d = nc.dram_tensor("out", [QS, C], BF16, kind="ExternalOutput")

    with tile.TileContext(nc) as tc, ExitStack() as top:
        consts = top.enter_context(tc.tile_pool(name="consts", bufs=1))
        pxt = top.enter_context(tc.tile_pool(name="pxt", bufs=1))
        pv = top.enter_context(tc.tile_pool(name="pv", bufs=1))
        pq = top.enter_context(tc.tile_pool(name="pq", bufs=1))
        pres = top.enter_context(tc.tile_pool(name="pres", bufs=1))
        pmisc = top.enter_context(tc.tile_pool(name="pmisc", bufs=1))
        pe = top.enter_context(tc.tile_pool(name="pe", bufs=5))
        pef = top.enter_context(tc.tile_pool(name="pef", bufs=4))
        # PSUM: sc 2 + at 4 + rows 1 + ops 1 = 8 banks
        pss = top.enter_context(tc.tile_pool(name="pss", bufs=2, space="PSUM"))
        psat = top.enter_context(tc.tile_pool(name="psat", bufs=1, space="PSUM"))
        psr = top.enter_context(tc.tile_pool(name="psr", bufs=1, space="PSUM"))
        pso = top.enter_context(tc.tile_pool(name="pso", bufs=1, space="PSUM"))

        # ---------- consts (no DMA dependencies) ----------
        one11 = consts.tile([1, 1], F32, name="one11")
        nc.sync.dma_start(out=one11, in_=one_d[:])
        # rowsum weight: folds the ATS*S3 denominator scale into the
        # partition reduction
        sumw = consts.tile([P, 1], F16, name="sumw")
        nc.vector.memset(sumw, ATS * S3)
        sumw8 = consts.tile([P, 1], FP8, name="sumw8")
        nc.vector.memset(sumw8, ATS * S3)
        ebias = consts.tile([P, 1], F32, name="ebias")
        nc.vector.memset(ebias, SHIFT)
        warm32 = pmisc.tile([P, 512], F32, name="warm32")
        nc.vector.memset(warm32, 1.0)
        warmr = pmisc.tile([P, 512], mybir.dt.float32r, name="warmr")
        nc.vector.tensor_copy(warmr, warm32)

        # ---------- resident tensors ----------
        xat8 = [pxt.tile([P, 8, 2, 512], FP8, name=f"xat8_{g}",
                         tag=f"xat8_{g}") for g in range(CP)]
        xnat = [pv.tile([P, 2, C], FP8, name=f"xnat_{g}", tag=f"xnat_{g}")
                for g in range(NP2)]
        xaq8 = pq.tile([P, QCH, 2 * CP, 512], FP8, name="xaq8", tag="xaq8")
        u8 = [pq.tile([P, 2, QS], FP8, name=f"u8_{g}", tag=f"u8_{g}")
              for g in range(CP)]
        w2T8 = pq.tile([P, 2, CP * C], FP8, name="w2T8", tag="w2T8")
        w38 = pq.tile([P, 2, CP * C], FP8, name="w38", tag="w38")
        res16 = [pres.tile([P, C], F16, name=f"res16_{i}", tag=f"res_{i}")
                 for i in range(QTILES)]
        # c2 ships as a single row (one DMA packet; a [P,1]-shaped load is
        # 128 four-byte packets that clog the startup DMA window) and is
        # transposed on-device via tiny one11 matmuls.
        c2row = consts.tile([1, C], F32, name="c2row")
        nc.sync.dma_start(out=c2row, in_=c2_d[:])

        # ---------- DMA issue order: the ~16 hardware queues are assigned
        # round-robin in call order and run CONCURRENTLY, so a transfer's
        # priority is its share of queues, not its position.  Slice the
        # u8-projection dependencies (w2T8 + xaq8) along the PARTITION dim
        # (keeps per-row bursts contiguous; free-dim slicing fragments rows
        # into tiny bursts) so they own most of the queues and land first;
        # everything else follows in consumption order, residuals last. ----
        nc.sync.dma_start(out=w2T8, in_=w2T8_d[:])
        for ch2 in range(QCH):
            nc.sync.dma_start(out=xaq8[:, ch2], in_=xaq8_d[:, ch2])
        nc.sync.dma_start(out=w38, in_=w38_d[:])
        for ch in range(8):          # 512-pixel column chunks, kt-major
            for g in range(CP):
                nc.sync.dma_start(out=xat8[g][:, ch], in_=xat8_d[g][ch])
            nc.sync.dma_start(out=xnat[2 * ch], in_=xnat_d[2 * ch])
            nc.sync.dma_start(out=xnat[2 * ch + 1], in_=xnat_d[2 * ch + 1])
        for i in range(QTILES):
            nc.sync.dma_start(out=res16[i], in_=res_d[i * P:(i + 1) * P, :])

        def keep_warm(n):
            # Full-width f32r matmuls keep the HAM clock at full rate while
            # the PE would otherwise idle (low-toggle fp8 matmuls don't
            # register enough activity and the whole core drops to half
            # clock, with ~10us of hysteresis).
            for _ in range(n):
                wps = pss.tile([P, 512], F32, name="wps", tag="sc")
                nc.tensor.matmul(wps, lhsT=warmr[:, 0:P], rhs=warmr,
                                 start=True, stop=True)

        keep_warm(2)

        # Preload the scalar engine's Exp activation table (costs 1.3us;
        # otherwise it lands at the first real exp, right at attention
        # start).  The scalar engine runs nothing but Exp, so the table is
        # never evicted.
        expwarm = pmisc.tile([P, 1], F32, name="expwarm")
        nc.scalar.activation(expwarm, ebias, AF.Exp, bias=ebias, scale=1.0)

        # c2 row -> per-partition [P, 4] via tiny transpose matmuls
        c2_ps = pso.tile([P, 8], F32, name="c2_ps", tag="ops")
        for ct in range(CT):
            nc.tensor.matmul(c2_ps[:, ct:ct + 1],
                             lhsT=c2row[0:1, ct * P:(ct + 1) * P],
                             rhs=one11, start=True, stop=True)
        c24 = pmisc.tile([P, 4], F32, name="c24")
        nc.vector.tensor_copy(c24, c2_ps[:, 0:4])
        keep_warm(1)

        # ---- query projection: u = W2^T xa_q / S2 + c2, fp8 ----
        # ps lives in the pso bank: chunk 1's projections are injected into
        # chunk 0's score stream, and borrowing the sc rotation there would
        # stall the scores on the exp reads.
        def u8_proj(ch2, ci_t):
            if ci_t % 2 == 0:
                ps = pso.tile([P, 512], F32, name="ups", tag="ops")
            else:
                ps = psr.tile([P, 512], F32, name="ups2", tag="rows")
            for gq in range(CP):
                nc.tensor.matmul(
                    ps,
                    lhsT=w2T8[:, :, gq * C + ci_t * P:
                              gq * C + (ci_t + 1) * P],
                    rhs=xaq8[:, ch2, 2 * gq:2 * gq + 2, :],
                    start=(gq == 0), stop=(gq == CP - 1),
                    perf_mode=DR)
            og, oi = divmod(ci_t, 2)
            nc.vector.tensor_scalar(
                u8[og][:, oi, ch2 * 512:(ch2 + 1) * 512],
                ps, 1.0 / S2, c24[:, ci_t:ci_t + 1], ALU.mult, ALU.add)

        for ci_t in range(CT):
            u8_proj(0, ci_t)

        # ---- attention + output projection ----
        # ep_carry: thunks of deferred PE/DVE work (the previous chunk's
        # epilogue, or the second chunk's query projection) injected one
        # per score group so the in-order PE queue never stalls on the
        # DVE-paced epilogue.  The chunk's first group has no attn_v, so
        # multi-matmul thunks land there for free.
        ep_carry = [(lambda ci_t=ci_t: u8_proj(1, ci_t)) for ci_t in range(CT)]

        def attn_v(g, probs, at_ps):
            for co in range(CT):
                nc.tensor.matmul(
                    at_ps[co],
                    lhsT=xnat[g][:, :, co * P:(co + 1) * P],
                    rhs=probs,
                    start=(g == 0), stop=(g == NP2 - 1),
                    perf_mode=DR)

        for qc in range(QCH):
            at_ps = [psat.tile([P, 512], F32, name=f"at{co}",
                               tag=f"at{co}") for co in range(CT)]
            # Softmax denominators, split: even groups' probs accumulate on
            # the DVE into acc (f16), odd groups' via a ones-matmul on the
            # PE into rows_ps.  All-DVE backs up the DVE queue and stalls
            # the probs rotation; all-PE costs a 9th matmul every group.
            acc = pe.tile([P, 2, 512], F16, name="acc", tag="acc")

            prev = None
            for g in range(NP2):
                scs = []
                for j in range(2):
                    kt_i = 2 * g + j
                    sc = pss.tile([P, 512], F32, name="sc", tag="sc")
                    kch, kof = divmod(kt_i, 4)
                    for c in range(CP):
                        nc.tensor.matmul(
                            sc,
                            lhsT=xat8[c][:, kch, :,
                                         kof * P:(kof + 1) * P],
                            rhs=u8[c][:, :, qc * 512:(qc + 1) * 512],
                            start=(c == 0), stop=(c == CP - 1),
                            perf_mode=DR)
                    scs.append(sc)
                if prev is not None:
                    attn_v(g - 1, prev, at_ps)
                if ep_carry:
                    ep_carry.pop(0)()
                probs = pe.tile([P, 2, 512], FP8, name="probs", tag="probs")
                for j in range(2):
                    nc.scalar.activation(probs[:, j, :], scs[j], AF.Exp,
                                         bias=ebias, scale=ISQ)
                if g == 0:
                    nc.vector.tensor_copy(acc, probs)
                elif g < NP2 - 1:
                    # group 15 is folded into do_rows directly from its
                    # probs tile, so the denominators never wait on the
                    # final DVE accumulate
                    nc.vector.tensor_add(acc, acc, probs)
                prev = probs
            attn_v(NP2 - 1, prev, at_ps)

            # ---- chunk epilogue ----
            last = qc == QCH - 1
            z8 = [pe.tile([P, 2, 512], FP8, name=f"z8_{zg}", tag=f"z8_{zg}")
                  for zg in range(CP)]
            recq4 = pe.tile([P, 4], F32, name="recq4", tag="recq4")

            def do_rows(acc=acc, recq4=recq4, p15=prev):
                # denominators directly in partition layout: one N=1 matmul
                # per (tile, pair) over the f16 accumulator (groups 0-14)
                # plus the last group's fp8 probs -- no [32, 512] reduction,
                # no transposes, one reciprocal
                rq_ps = pso.tile([P, 8], F32, name="rq_ps", tag="ops")
                for qt in range(4):
                    for i in range(2):
                        nc.tensor.matmul(
                            rq_ps[:, qt:qt + 1],
                            lhsT=acc[:, i, qt * P:(qt + 1) * P],
                            rhs=sumw, start=(i == 0), stop=False)
                    for i in range(2):
                        nc.tensor.matmul(
                            rq_ps[:, qt:qt + 1],
                            lhsT=p15[:, i, qt * P:(qt + 1) * P],
                            rhs=sumw8, start=False, stop=(i == 1))
                nc.vector.reciprocal(recq4, rq_ps[:, 0:4])

            def mk_oproj(qt, qc=qc, z8=z8, recq4=recq4, opool=None,
                         fin_eng=None):
                def thunk():
                    if opool is None:
                        ops = pso.tile([P, C], F32, name="ops", tag="ops")
                    else:
                        ops = opool[0].tile([P, C], F32, name="ops",
                                            tag=opool[1])
                    for zg in range(CP):
                        nc.tensor.matmul(
                            ops, lhsT=z8[zg][:, :, qt * P:(qt + 1) * P],
                            rhs=w38[:, :, zg * C:(zg + 1) * C],
                            start=(zg == 0),
                            stop=(zg == CP - 1), perf_mode=DR)
                    fin2 = pef.tile([P, C], BF16, name="fin2", tag="fin2")
                    (fin_eng or nc.vector).scalar_tensor_tensor(
                        fin2, ops, recq4[:, qt:qt + 1], res16[qc * 4 + qt],
                        ALU.mult, ALU.add)
                    r0 = (qc * 4 + qt) * P
                    nc.sync.dma_start(out=out_d[r0:r0 + P, :], in_=fin2)
                return thunk

            if qc < QCH - 1:
                for ci_t in range(CT):
                    og, oi = divmod(ci_t, 2)
                    nc.vector.tensor_scalar_mul(z8[og][:, oi, :],
                                                at_ps[ci_t], ATS)
                ep_carry = [do_rows, mk_oproj(0, opool=(psr, "rows"))] + \
                    [mk_oproj(qt) for qt in range(1, 4)]
            else:
                # Last chunk: no next score stream to hide behind.  Spread
                # the out-projections over the now-free sc/at banks so they
                # run back-to-back, with warm matmuls keeping the clock up
                # through the fin/DMA drain.  z8 rides the freed scalar
                # engine; its first op takes a zero bias derived from recq4
                # -- a fake dependency that makes the denominator matmuls
                # transitively critical, so the dependency-order scheduler
                # runs them before the out-projection chain instead of
                # after it (which stalls the fins on the reciprocal).
                do_rows()
                sats = pe.tile([P, 1], F32, name="sats", tag="sats")
                nc.vector.tensor_scalar(sats, recq4[:, 0:1], 0.0, ATS,
                                        ALU.mult, ALU.add)
                for ci_t in range(CT):
                    og, oi = divmod(ci_t, 2)
                    if ci_t == 0:
                        nc.scalar.activation(z8[og][:, oi, :], at_ps[ci_t],
                                             AF.Copy, bias=0.0, scale=sats)
                    else:
                        nc.scalar.mul(z8[og][:, oi, :], at_ps[ci_t], ATS)
                keep_warm(2)
                mk_oproj(0, opool=(psr, "rows"))()
                mk_oproj(1, opool=(pss, "sc"))()
                keep_warm(2)
                mk_oproj(2, opool=(pss, "sc"))()
                mk_oproj(3, opool=(psat, "at0"))()
                keep_warm(2)

    nc.compile()
    return nc


def _make_in_maps(inputs):
    import ml_dtypes
    FP8NP = ml_dtypes.float8_e4m3
    x = np.ascontiguousarray(np.asarray(inputs["inputs"], dtype=np.float32))
    xf = x.reshape(B, N, C)
    gamma = np.asarray(inputs["gn_gamma"], np.float32)
    beta = np.asarray(inputs["gn_beta"], np.float32)
    wq = np.asarray(inputs["wq"], np.float32)
    wk = np.asarray(inputs["wk"], np.float32)
    wv = np.asarray(inputs["wv"], np.float32)
    wo = np.asarray(inputs["wo"], np.float32)
    bq = np.asarray(inputs["bq"], np.float32)
    bv = np.asarray(inputs["bv"], np.float32)
    bo = np.asarray(inputs["bo"], np.float32)

    shared = {"one11": np.ones((1, 1), np.float32)}
    w2T = (wq @ wk.T) * S2
    w2T8p = w2T.astype(FP8NP).reshape(CP, 2, P, C).transpose(2, 1, 0, 3)
    shared["w2T8c"] = np.ascontiguousarray(
        w2T8p.reshape(P, 2, CP * C))
    w3 = (wv @ wo) * S3
    w38p = w3.astype(FP8NP).reshape(CP, 2, P, C).transpose(2, 1, 0, 3)
    shared["w38c"] = np.ascontiguousarray(w38p.reshape(P, 2, CP * C))

    # Per-batch GroupNorm folds.
    per_b = []
    for b in range(B):
        xg = xf[b].reshape(N, G, GS)
        mean = xg.mean(axis=(0, 2))
        var = xg.var(axis=(0, 2))
        a = (gamma.reshape(G, GS) / np.sqrt(var[:, None] + EPS)).reshape(C)
        bvec = beta - np.repeat(mean, GS) * a
        xa = xf[b] * a                               # [N, C]
        xa8 = xa.astype(FP8NP)
        # channel-pair tiles: xat8p[g][p, i, n] = xa^T[g*256 + i*128 + p, n]
        xaT = np.ascontiguousarray(xa8.T)            # [C, N] fp8
        xat_pairs = [np.ascontiguousarray(
            xaT.reshape(CP, 2, P, N)[g].transpose(1, 0, 2))
            for g in range(CP)]                          # [P, 2, N]
        # chunk-major DMA view: [8, P, 2, 512]
        xat_cm = [np.ascontiguousarray(
            t.reshape(P, 2, 8, 512).transpose(2, 0, 1, 3))
            for t in xat_pairs]
        # pixel-pair tiles: xnat8[gk][p, ik, c]
        xnat = np.ascontiguousarray(
            xa8.reshape(NP2, 2, P, C).transpose(0, 2, 1, 3))
        c2v = wk @ (bvec @ wq + bq)                  # [C]
        bo_eff = (bvec @ wv + bv) @ wo + bo          # [C]
        per_b.append((xat_pairs, xat_cm, xnat,
                      np.ascontiguousarray(c2v.astype(np.float32)
                                           .reshape(1, C)), bo_eff))

    in_maps = []
    for core in range(NCORES):
        b, qq = divmod(core, 4)
        xat_pairs, xat_cm, xnat, c2v, bo_eff = per_b[b]
        m = dict(shared)
        for g in range(CP):
            m[f"xat8p{g}"] = xat_cm[g]
        m["xnat8"] = xnat
        m["c2"] = c2v
        # [P, 2*CP, QS] -> chunk-major [P, QCH, 2*CP, 512]
        xq = np.concatenate(
            [xat_pairs[g][:, :, qq * QS:(qq + 1) * QS] for g in range(CP)],
            axis=1)
        m["xaq8"] = np.ascontiguousarray(
            xq.reshape(P, 2 * CP, QCH, 512).transpose(0, 2, 1, 3))
        m["res16"] = np.ascontiguousarray(
            (xf[b, qq * QS:(qq + 1) * QS, :] + bo_eff).astype(np.float16))
        in_maps.append(m)
    return in_maps


def _assemble(results):
    out = np.empty((B, N, C), dtype=np.float32)
    for core in range(NCORES):
        b, qq = divmod(core, 4)
        out[b, qq * QS:(qq + 1) * QS, :] = results[core]["out"]
    return out.reshape(B, HH, WW, C)


def kernel(**inputs):
    global _NC_CACHE
    if _NC_CACHE is None:
        _NC_CACHE = _build()
    in_maps = _make_in_maps(inputs)
    res = run_bass_kernel_spmd(_NC_CACHE, in_maps, list(range(NCORES)))
    return _assemble(res.results)


def _install_ntff_shim():
    """The agent image's antenv lacks axon_hooks; provide it so
    run_bass_kernel_spmd(trace=True) can NTFF-profile through axon."""
    import types
    import antenv
    if "antenv.axon_hooks" in sys.modules:
        return
    mod = types.ModuleType("antenv.axon_hooks")
    mod._hook = None

    def set_axon_ntff_profile_hook(h):
        mod._hook = h

    def get_axon_ntff_profile_hook():
        return mod._hook

    mod.set_axon_ntff_profile_hook = set_axon_ntff_profile_hook
    mod.get_axon_ntff_profile_hook = get_axon_ntff_profile_hook
    sys.modules["antenv.axon_hooks"] = mod
    antenv.axon_hooks = mod
    sys.path.insert(0, "/root/.axon_site")
    from trn_agent_boot.trn_boot import _ntff_profile_via_ctypes
    hook = _ntff_profile_via_ctypes("/opt/axon/libaxon_pjrt.so")
    set_axon_ntff_profile_hook(hook)


def run_traced(inputs, trace_kwargs=None):
    """Traced run for profiling: returns (BassKernelResults, tmpdir)."""
    global _NC_CACHE
    if _NC_CACHE is None:
        _NC_CACHE = _build()
    import tempfile
    _install_ntff_shim()
    in_maps = _make_in_maps(inputs)
    tmpdir = tempfile.mkdtemp(prefix="trace_")
    res = run_bass_kernel_spmd(_NC_CACHE, in_maps, list(range(NCORES)),
                               trace=True, tmpdir=tmpdir,
                               trace_kwargs=trace_kwargs or {})
    return res, tmpdir
